# revision 1
# baseline (speedup 1.0000x reference)
"""DetectionCriterion loss kernel for Trainium2 (8 NeuronCores, data-parallel over batch).

Strategy:
  - Shard batch B=16 over 8 cores (2 batches/core).
  - Dense heatmap focal loss is computed as "all-negative" focal plus sparse
    corrections at the ~800 scattered positive points:
        focal0(x) = 0.75 * softplus(x) * sigmoid(x)^2
    computed with a single ACT table set (natural_log_exp):
        u = exp(-x); n = ln(1+u) (= softplus(-x)); w = exp(-2n) (= sigmoid(x)^2)
        focal0 = 0.75 * (x + n) * w
    Sparse corrections / CE class targets / matched box pairs are pure gathers
    of raw input rows (index plumbing done host-side, all arithmetic on device).
  - Each core emits 8 partial sums; host all-reduces and does final divisions.
"""

import os
import numpy as np
from contextlib import ExitStack

# No NTFF hook exists in this container; a stray BASS_TRACE=1 would crash
# run_bass_kernel_spmd on an antenv.axon_hooks import.
os.environ["BASS_NEVER_TRACE"] = "1"

# ---- problem constants (hardcoded from the nn_DetectionCriterion spec) ----
B, Q, C1 = 16, 300, 81          # batch, queries, classes+1
C = 80                          # num classes
T = 50                          # targets per batch
H = W = 128                     # heatmap spatial
NCORES = 8
BL = B // NCORES                # batches per core = 2
NUM_CLASSES = 80

W_CE, W_BBOX, W_GIOU = 1.0, 5.0, 2.0
AUX_W, AUX_HM_W, AUX_BOX_W = 1.0, 1.0, 5.0

HM_ELEMS = BL * C * H * W       # 2,621,440 per core
HM_F = HM_ELEMS // 128          # 20480
HM_TILE = 2048
HM_NT = HM_F // HM_TILE         # 10

ROWS = BL * Q                   # 600 logit rows per core
LG_NT = 5
ROWS_PAD = LG_NT * 128          # 640

NPAIR = BL * T                  # 100 matched pairs per core
SP = 128                        # padded sparse rows (one per partition)

NCOL = 8                        # per-core output columns:
# 0: hm dense focal0 sum   1: hm sparse correction sum
# 2: ce numerator          3: ce weight sum
# 4: bbox L1 sum           5: (1-giou) sum
# 6: box-map L1 sum        7: num_pos

_CACHE = {}
LAST_RESULTS = None  # BassKernelResults of last run (for profiling in test.py)


def _build_module(nrep=1, variant="v1"):
    import concourse.bass as bass
    from concourse import bacc, mybir
    import concourse.tile as tile

    AF = mybir.ActivationFunctionType
    OP = mybir.AluOpType
    AX = mybir.AxisListType
    f32 = mybir.dt.float32
    bf16 = mybir.dt.bfloat16

    nc = bacc.Bacc(
        "TRN2",
        target_bir_lowering=False,
        debug=False,
        enable_asserts=False,
        num_devices=NCORES,
    )

    hm_d = nc.dram_tensor("hm", [128, HM_F], f32, kind="ExternalInput")
    lg_d = nc.dram_tensor("lg", [ROWS_PAD, C1], f32, kind="ExternalInput")
    sel_d = nc.dram_tensor("sel", [ROWS_PAD, C1], f32, kind="ExternalInput")
    cw_d = nc.dram_tensor("cw", [ROWS_PAD], f32, kind="ExternalInput")
    srcb_d = nc.dram_tensor("srcb", [SP, 4], f32, kind="ExternalInput")
    tgtb_d = nc.dram_tensor("tgtb", [SP, 4], f32, kind="ExternalInput")
    sclb_d = nc.dram_tensor("sclb", [SP, 4], f32, kind="ExternalInput")
    hmx_d = nc.dram_tensor("hmx", [SP, 1], f32, kind="ExternalInput")
    hmw_d = nc.dram_tensor("hmw", [SP, 1], f32, kind="ExternalInput")
    bxv_d = nc.dram_tensor("bxv", [SP, 4], f32, kind="ExternalInput")
    bxt_d = nc.dram_tensor("bxt", [SP, 4], f32, kind="ExternalInput")
    bxs_d = nc.dram_tensor("bxs", [SP, 4], f32, kind="ExternalInput")
    bxw_d = nc.dram_tensor("bxw", [SP, 1], f32, kind="ExternalInput")
    out_d = nc.dram_tensor("out", [1, NCOL], f32, kind="ExternalOutput")

    with tile.TileContext(nc) as tc, ExitStack() as ctx:
        xp = ctx.enter_context(tc.tile_pool(name="xp", bufs=3))
        up = ctx.enter_context(tc.tile_pool(name="up", bufs=3))
        npool = ctx.enter_context(tc.tile_pool(name="npool", bufs=3))
        wp = ctx.enter_context(tc.tile_pool(name="wp", bufs=3))
        jp = ctx.enter_context(tc.tile_pool(name="jp", bufs=2))
        jq = ctx.enter_context(tc.tile_pool(name="jq", bufs=2))
        sm = ctx.enter_context(tc.tile_pool(name="sm", bufs=1))
        ps = ctx.enter_context(tc.tile_pool(name="ps", bufs=1, space="PSUM"))

        def _one_rep():
            acc = sm.tile([128, NCOL], f32, tag="acc")

            # ---------------- dense heatmap focal (all-negative) ----------------
            hm_parts = sm.tile([128, 2 * HM_NT], f32, tag="hm_parts")
            hm_ap = hm_d.ap()
            if variant == "v1":
                for i in range(HM_NT):
                    x = xp.tile([128, HM_TILE], f32, tag="x")
                    nc.sync.dma_start(x[:], hm_ap[:, i * HM_TILE:(i + 1) * HM_TILE])
                    u = up.tile([128, HM_TILE], f32, tag="u")
                    nc.scalar.activation(u[:], x[:], AF.Exp, scale=-1.0)
                    n = npool.tile([128, HM_TILE], f32, tag="n")
                    nc.scalar.activation(n[:], u[:], AF.Ln, bias=1.0)
                    w = wp.tile([128, HM_TILE], f32, tag="w")
                    nc.scalar.activation(w[:], n[:], AF.Exp, scale=-2.0)
                    j1 = jp.tile([128, HM_TILE], f32, tag="j1")
                    nc.vector.scalar_tensor_tensor(
                        j1[:], x[:], 0.75, w[:], op0=OP.mult, op1=OP.mult,
                        accum_out=hm_parts[:, 2 * i:2 * i + 1])
                    j2 = jq.tile([128, HM_TILE], f32, tag="j2")
                    nc.vector.scalar_tensor_tensor(
                        j2[:], n[:], 0.75, w[:], op0=OP.mult, op1=OP.mult,
                        accum_out=hm_parts[:, 2 * i + 1:2 * i + 2])
            elif variant == "dma":
                for i in range(HM_NT):
                    x = xp.tile([128, HM_TILE], f32, tag="x")
                    nc.sync.dma_start(x[:], hm_ap[:, i * HM_TILE:(i + 1) * HM_TILE])
                    nc.vector.tensor_reduce(
                        hm_parts[:, 2 * i:2 * i + 1], x[:, 0:4], axis=AX.X,
                        op=OP.add)
                    nc.vector.tensor_reduce(
                        hm_parts[:, 2 * i + 1:2 * i + 2], x[:, 4:8], axis=AX.X,
                        op=OP.add)
            elif variant == "v2":
                # g = 0.75*(x - ln(sigmoid(x))) * sigmoid(x)^2, two ACT passes.
                # Phase A: all sigmoids (sigmoid table set); Phase B: all Ln
                # (natural_log set) + products. s stored bf16.
                xs = []
                ss = []
                for i in range(HM_NT):
                    x = sm.tile([128, HM_TILE], f32, tag=f"x{i}")
                    nc.sync.dma_start(x[:], hm_ap[:, i * HM_TILE:(i + 1) * HM_TILE])
                    s = sm.tile([128, HM_TILE], bf16, tag=f"s{i}")
                    nc.scalar.activation(s[:], x[:], AF.Sigmoid)
                    xs.append(x)
                    ss.append(s)
                tc.no_sync_barrier()
                for i in range(HM_NT):
                    x, s = xs[i], ss[i]
                    ll = npool.tile([128, HM_TILE], bf16, tag="ll")
                    nc.scalar.activation(ll[:], s[:], AF.Ln)
                    m = wp.tile([128, HM_TILE], bf16, tag="m")
                    nc.vector.tensor_mul(m[:], s[:], s[:])
                    j1 = jp.tile([128, HM_TILE], f32, tag="j1")
                    nc.vector.scalar_tensor_tensor(
                        j1[:], x[:], 0.75, m[:], op0=OP.mult, op1=OP.mult,
                        accum_out=hm_parts[:, 2 * i:2 * i + 1])
                    j2 = jq.tile([128, HM_TILE], f32, tag="j2")
                    nc.vector.scalar_tensor_tensor(
                        j2[:], ll[:], -0.75, m[:], op0=OP.mult, op1=OP.mult,
                        accum_out=hm_parts[:, 2 * i + 1:2 * i + 2])
            nc.vector.tensor_reduce(acc[:, 0:1], hm_parts[:], axis=AX.X, op=OP.add)

            # ---------------- CE (weighted log-softmax NLL) ----------------
            lg_all = sm.tile([128, LG_NT * C1], f32, tag="lg_all")
            nc.sync.dma_start(
                lg_all[:].rearrange("p (t c) -> p t c", t=LG_NT),
                lg_d.ap().rearrange("(t p) c -> p t c", p=128))
            sel_all = sm.tile([128, LG_NT * C1], f32, tag="sel_all")
            nc.sync.dma_start(
                sel_all[:].rearrange("p (t c) -> p t c", t=LG_NT),
                sel_d.ap().rearrange("(t p) c -> p t c", p=128))
            cw_all = sm.tile([128, LG_NT], f32, tag="cw_all")
            nc.sync.dma_start(cw_all[:], cw_d.ap().rearrange("(t p) -> p t", p=128))

            nmx = sm.tile([128, LG_NT], f32, tag="nmx")
            se = sm.tile([128, LG_NT], f32, tag="se")
            lnse = sm.tile([128, LG_NT], f32, tag="lnse")
            tsum = sm.tile([128, LG_NT], f32, tag="tsum")
            d_all = sm.tile([128, LG_NT], f32, tag="d_all")
            for i in range(LG_NT):
                lg_i = lg_all[:, i * C1:(i + 1) * C1]
                nc.vector.tensor_reduce(
                    nmx[:, i:i + 1], lg_i, axis=AX.X, op=OP.max, negate=True)
                e_i = jq.tile([128, C1], f32, tag="e_i")
                nc.scalar.activation(
                    e_i[:], lg_i, AF.Exp, bias=nmx[:, i:i + 1], scale=1.0,
                    accum_out=se[:, i:i + 1])
                nc.scalar.activation(lnse[:, i:i + 1], se[:, i:i + 1], AF.Ln)
                j3 = jq.tile([128, C1], f32, tag="j3")
                nc.vector.scalar_tensor_tensor(
                    j3[:], lg_i, 1.0, sel_all[:, i * C1:(i + 1) * C1],
                    op0=OP.mult, op1=OP.mult, accum_out=tsum[:, i:i + 1])
                # d = (ln(sum e) - (-max)) - t  = lse - x[tc]
                nc.vector.scalar_tensor_tensor(
                    d_all[:, i:i + 1], lnse[:, i:i + 1], nmx[:, i:i + 1],
                    tsum[:, i:i + 1], op0=OP.subtract, op1=OP.subtract)
            jce = sm.tile([128, LG_NT], f32, tag="jce")
            nc.vector.scalar_tensor_tensor(
                jce[:], d_all[:], 1.0, cw_all[:],
                op0=OP.mult, op1=OP.mult, accum_out=acc[:, 2:3])
            nc.vector.tensor_reduce(acc[:, 3:4], cw_all[:], axis=AX.X, op=OP.add)

            # ---------------- sparse heatmap corrections ----------------
            # corr = w * (0.25*g(-x) - 0.75*g(x)),  g(x) = (x + n(x)) * exp(-2 n(x))
            hx = sm.tile([128, 1], f32, tag="hx")
            nc.sync.dma_start(hx[:], hmx_d.ap())
            hw_ = sm.tile([128, 1], f32, tag="hw_")
            nc.sync.dma_start(hw_[:], hmw_d.ap())

            def g_of(x_ap, sgn, tagp):
                u1 = sm.tile([128, 1], f32, tag=f"{tagp}u")
                nc.scalar.activation(u1[:], x_ap, AF.Exp, scale=-1.0 * sgn)
                n1 = sm.tile([128, 1], f32, tag=f"{tagp}n")
                nc.scalar.activation(n1[:], u1[:], AF.Ln, bias=1.0)
                w1 = sm.tile([128, 1], f32, tag=f"{tagp}w")
                nc.scalar.activation(w1[:], n1[:], AF.Exp, scale=-2.0)
                t1 = sm.tile([128, 1], f32, tag=f"{tagp}t")
                if sgn > 0:
                    nc.vector.tensor_add(t1[:], hx[:], n1[:])
                else:
                    nc.vector.tensor_sub(t1[:], n1[:], hx[:])
                g1 = sm.tile([128, 1], f32, tag=f"{tagp}g")
                nc.vector.tensor_mul(g1[:], t1[:], w1[:])
                return g1

            g_pos = g_of(hx[:], +1, "gp")   # g(x)
            g_neg = g_of(hx[:], -1, "gn")   # g(-x)
            g1s = sm.tile([128, 1], f32, tag="g1s")
            nc.vector.tensor_scalar_mul(g1s[:], g_pos[:], 0.75)
            mcor = sm.tile([128, 1], f32, tag="mcor")
            nc.vector.scalar_tensor_tensor(
                mcor[:], g_neg[:], 0.25, g1s[:], op0=OP.mult, op1=OP.subtract)
            nc.vector.tensor_mul(acc[:, 1:2], mcor[:], hw_[:])

            # ---------------- matched box pairs: L1 + GIoU ----------------
            src = sm.tile([SP, 4], f32, tag="src")
            nc.sync.dma_start(src[:], srcb_d.ap())
            tgt = sm.tile([SP, 4], f32, tag="tgt")
            nc.sync.dma_start(tgt[:], tgtb_d.ap())
            scl = sm.tile([SP, 4], f32, tag="scl")
            nc.sync.dma_start(scl[:], sclb_d.ap())

            rsc = sm.tile([SP, 4], f32, tag="rsc")
            nc.vector.reciprocal(rsc[:], scl[:])
            tn = sm.tile([SP, 4], f32, tag="tn")
            nc.vector.tensor_mul(tn[:], tgt[:], rsc[:])          # xyxy normalized
            th = sm.tile([SP, 4], f32, tag="th")
            nc.vector.tensor_scalar_mul(th[:], tn[:], 0.5)
            tcc = sm.tile([SP, 4], f32, tag="tcc")               # cxcywh normalized
            nc.vector.tensor_add(tcc[:, 0:1], th[:, 0:1], th[:, 2:3])
            nc.vector.tensor_add(tcc[:, 1:2], th[:, 1:2], th[:, 3:4])
            nc.vector.tensor_sub(tcc[:, 2:3], tn[:, 2:3], tn[:, 0:1])
            nc.vector.tensor_sub(tcc[:, 3:4], tn[:, 3:4], tn[:, 1:2])
            dif = sm.tile([SP, 4], f32, tag="dif")
            nc.vector.tensor_sub(dif[:], src[:], tcc[:])
            nc.vector.tensor_reduce(
                acc[:, 4:5], dif[:], axis=AX.X, op=OP.add, apply_absolute_value=True)

            # src cxcywh -> xyxy
            sh = sm.tile([SP, 4], f32, tag="sh")
            nc.vector.tensor_scalar_mul(sh[:], src[:], 0.5)
            sxy = sm.tile([SP, 4], f32, tag="sxy")
            nc.vector.tensor_sub(sxy[:, 0:1], src[:, 0:1], sh[:, 2:3])
            nc.vector.tensor_sub(sxy[:, 1:2], src[:, 1:2], sh[:, 3:4])
            nc.vector.tensor_add(sxy[:, 2:3], src[:, 0:1], sh[:, 2:3])
            nc.vector.tensor_add(sxy[:, 3:4], src[:, 1:2], sh[:, 3:4])

            aa = sm.tile([SP, 1], f32, tag="aa")
            nc.vector.tensor_mul(aa[:], src[:, 2:3], src[:, 3:4])
            ab = sm.tile([SP, 1], f32, tag="ab")
            nc.vector.tensor_mul(ab[:], tcc[:, 2:3], tcc[:, 3:4])

            mx1 = sm.tile([SP, 1], f32, tag="mx1")
            nc.vector.tensor_max(mx1[:], sxy[:, 0:1], tn[:, 0:1])
            my1 = sm.tile([SP, 1], f32, tag="my1")
            nc.vector.tensor_max(my1[:], sxy[:, 1:2], tn[:, 1:2])
            nx2 = sm.tile([SP, 1], f32, tag="nx2")
            nc.vector.tensor_tensor(nx2[:], sxy[:, 2:3], tn[:, 2:3], op=OP.min)
            ny2 = sm.tile([SP, 1], f32, tag="ny2")
            nc.vector.tensor_tensor(ny2[:], sxy[:, 3:4], tn[:, 3:4], op=OP.min)

            wi = sm.tile([SP, 1], f32, tag="wi")
            nc.vector.tensor_sub(wi[:], nx2[:], mx1[:])
            nc.vector.tensor_scalar_max(wi[:], wi[:], 0.0)
            hi = sm.tile([SP, 1], f32, tag="hi")
            nc.vector.tensor_sub(hi[:], ny2[:], my1[:])
            nc.vector.tensor_scalar_max(hi[:], hi[:], 0.0)
            inter = sm.tile([SP, 1], f32, tag="inter")
            nc.vector.tensor_mul(inter[:], wi[:], hi[:])
            uni = sm.tile([SP, 1], f32, tag="uni")
            nc.vector.tensor_add(uni[:], aa[:], ab[:])
            nc.vector.tensor_sub(uni[:], uni[:], inter[:])

            ex1 = sm.tile([SP, 1], f32, tag="ex1")
            nc.vector.tensor_tensor(ex1[:], sxy[:, 0:1], tn[:, 0:1], op=OP.min)
            ey1 = sm.tile([SP, 1], f32, tag="ey1")
            nc.vector.tensor_tensor(ey1[:], sxy[:, 1:2], tn[:, 1:2], op=OP.min)
            ex2 = sm.tile([SP, 1], f32, tag="ex2")
            nc.vector.tensor_max(ex2[:], sxy[:, 2:3], tn[:, 2:3])
            ey2 = sm.tile([SP, 1], f32, tag="ey2")
            nc.vector.tensor_max(ey2[:], sxy[:, 3:4], tn[:, 3:4])
            cwe = sm.tile([SP, 1], f32, tag="cwe")
            nc.vector.tensor_sub(cwe[:], ex2[:], ex1[:])
            che = sm.tile([SP, 1], f32, tag="che")
            nc.vector.tensor_sub(che[:], ey2[:], ey1[:])
            ac_ = sm.tile([SP, 1], f32, tag="ac_")
            nc.vector.tensor_mul(ac_[:], cwe[:], che[:])

            runi = sm.tile([SP, 1], f32, tag="runi")
            nc.vector.reciprocal(runi[:], uni[:])
            rac = sm.tile([SP, 1], f32, tag="rac")
            nc.vector.reciprocal(rac[:], ac_[:])
            iou = sm.tile([SP, 1], f32, tag="iou")
            nc.vector.tensor_mul(iou[:], inter[:], runi[:])
            dac = sm.tile([SP, 1], f32, tag="dac")
            nc.vector.tensor_sub(dac[:], ac_[:], uni[:])
            t2_ = sm.tile([SP, 1], f32, tag="t2_")
            nc.vector.tensor_mul(t2_[:], dac[:], rac[:])
            vv = sm.tile([SP, 1], f32, tag="vv")
            nc.vector.tensor_sub(vv[:], t2_[:], iou[:])
            nc.vector.tensor_scalar_add(acc[:, 5:6], vv[:], 1.0)

            # ---------------- sparse box-map corrections ----------------
            bxv = sm.tile([SP, 4], f32, tag="bxv")
            nc.sync.dma_start(bxv[:], bxv_d.ap())
            bxt = sm.tile([SP, 4], f32, tag="bxt")
            nc.sync.dma_start(bxt[:], bxt_d.ap())
            bxs = sm.tile([SP, 4], f32, tag="bxs")
            nc.sync.dma_start(bxs[:], bxs_d.ap())
            bxw = sm.tile([SP, 1], f32, tag="bxw")
            nc.sync.dma_start(bxw[:], bxw_d.ap())

            rs2 = sm.tile([SP, 4], f32, tag="rs2")
            nc.vector.reciprocal(rs2[:], bxs[:])
            tnb = sm.tile([SP, 4], f32, tag="tnb")
            nc.vector.tensor_mul(tnb[:], bxt[:], rs2[:])
            tbh = sm.tile([SP, 4], f32, tag="tbh")
            nc.vector.tensor_scalar_mul(tbh[:], tnb[:], 0.5)
            bcc = sm.tile([SP, 4], f32, tag="bcc")
            nc.vector.tensor_add(bcc[:, 0:1], tbh[:, 0:1], tbh[:, 2:3])
            nc.vector.tensor_add(bcc[:, 1:2], tbh[:, 1:2], tbh[:, 3:4])
            nc.vector.tensor_sub(bcc[:, 2:3], tnb[:, 2:3], tnb[:, 0:1])
            nc.vector.tensor_sub(bcc[:, 3:4], tnb[:, 3:4], tnb[:, 1:2])
            dif2 = sm.tile([SP, 4], f32, tag="dif2")
            nc.vector.tensor_sub(dif2[:], bxv[:], bcc[:])
            ad2 = sm.tile([SP, 1], f32, tag="ad2")
            nc.vector.tensor_reduce(
                ad2[:], dif2[:], axis=AX.X, op=OP.add, apply_absolute_value=True)
            nc.vector.tensor_mul(acc[:, 6:7], ad2[:], bxw[:])
            nc.vector.tensor_copy(acc[:, 7:8], bxw[:])

            # ---------------- cross-partition reduce via PE ----------------
            ones = sm.tile([128, 1], f32, tag="ones")
            nc.vector.memset(ones[:], 1.0)
            pout = ps.tile([1, NCOL], f32, tag="pout")
            nc.tensor.matmul(pout[:], ones[:], acc[:], start=True, stop=True)
            outs = sm.tile([1, NCOL], f32, tag="outs")
            nc.vector.tensor_copy(outs[:], pout[:])
            nc.sync.dma_start(out_d.ap(), outs[:])

        for _rep in range(nrep):
            _one_rep()

    # Pin ACT table choice to the two sets that jointly cover
    # Sigmoid / Exp / Ln (+ fillers) — the default greedy per-function
    # choice alternates exp_and_others / natural_log and reloads tables
    # (~2.7us each) dozens of times per iteration.
    import types
    import bass_rust as _br
    from concourse.hw_specs import get_activation_tables

    def _pinned_insert_act_table_loads(self):
        has_activation = any(
            isinstance(i, mybir.InstActivation)
            for b in self.main_func.blocks
            for i in b.instructions
        )
        if not has_activation:
            return
        keep = {"sigmoid_and_others", "natural_log_exp_and_others"}
        tables = [
            (nm, (fs if nm in keep else set()))
            for nm, fs in get_activation_tables(self.m.arch).items()
        ]
        _br.insert_act_table_loads(self, tables)

    nc.insert_act_table_loads = types.MethodType(_pinned_insert_act_table_loads, nc)

    nc.compile()
    return nc


def _host_prepare(core, pred_logits, pred_boxes, heatmap_logits, box_map,
                  tgt_boxes, tgt_labels, tgt_sizes, src_idx, tgt_idx,
                  empty_weight):
    """Build the per-core input map. Only indexing/gather/padding on host."""
    f32 = np.float32
    bs = [BL * core + j for j in range(BL)]

    hm = np.ascontiguousarray(heatmap_logits[bs[0]:bs[-1] + 1]).reshape(128, HM_F)

    # CE: padded logits + one-hot select + class weights
    lg = np.zeros((ROWS_PAD, C1), f32)
    sel = np.zeros((ROWS_PAD, C1), f32)
    cw = np.zeros((ROWS_PAD,), f32)
    # matched box pairs
    srcb = np.zeros((SP, 4), f32)
    tgtb = np.zeros((SP, 4), f32)
    sclb = np.ones((SP, 4), f32)
    srcb[:, :] = np.array([0.5, 0.5, 0.5, 0.5], f32)
    tgtb[:, :] = np.array([160.0, 160.0, 480.0, 480.0], f32)
    sclb[:, :] = 640.0
    # sparse heatmap positives
    hmx = np.zeros((SP, 1), f32)
    hmw = np.zeros((SP, 1), f32)
    # sparse box-map cells
    bxv = np.zeros((SP, 4), f32)
    bxt = np.zeros((SP, 4), f32)
    bxt[:, :] = np.array([160.0, 160.0, 480.0, 480.0], f32)
    bxs = np.ones((SP, 4), f32)
    bxw = np.zeros((SP, 1), f32)

    hm_quads = {}   # (bloc, l, gy, gx) -> value
    cell_win = {}   # (bloc, gy, gx) -> winning target row j (last write wins)

    for j, b in enumerate(bs):
        lgb = pred_logits[b]                       # [Q, C1]
        lg[j * Q:(j + 1) * Q] = lgb
        tc_row = np.full((Q,), NUM_CLASSES, np.int64)
        ml = tgt_labels[b][tgt_idx[b]]             # matched labels
        tc_row[src_idx[b]] = ml
        sel[np.arange(Q) + j * Q, tc_row] = 1.0
        cw[j * Q:(j + 1) * Q] = empty_weight[tc_row]

        # matched pairs (in tgt_idx order, mirroring take_along_axis)
        srcb[j * T:(j + 1) * T] = pred_boxes[b][src_idx[b]]
        tgtb[j * T:(j + 1) * T] = tgt_boxes[b][tgt_idx[b]]
        h_im, w_im = tgt_sizes[b, 0], tgt_sizes[b, 1]
        svec = np.array([w_im, h_im, w_im, h_im], f32)
        sclb[j * T:(j + 1) * T] = svec

        # scatter positions from ALL targets in original order (f32 math
        # mirrors the reference exactly; used only to derive indices)
        tb = tgt_boxes[b].astype(f32)
        bn0 = (tb[:, 0] / svec[0] + tb[:, 2] / svec[2]) * f32(0.5)
        bn1 = (tb[:, 1] / svec[1] + tb[:, 3] / svec[3]) * f32(0.5)
        gx = np.clip((bn0 * f32(W)).astype(np.int32), 0, W - 1)
        gy = np.clip((bn1 * f32(H)).astype(np.int32), 0, H - 1)
        lf = tgt_labels[b]
        for t in range(T):
            hm_quads[(j, int(lf[t]), int(gy[t]), int(gx[t]))] = \
                heatmap_logits[b, lf[t], gy[t], gx[t]]
            cell_win[(j, int(gy[t]), int(gx[t]))] = t  # last occurrence wins

    # heatmap corrections
    for r, (k, v) in enumerate(hm_quads.items()):
        hmx[r, 0] = v
        hmw[r, 0] = 1.0

    # box-map corrections
    for r, ((j, gy, gx), t) in enumerate(cell_win.items()):
        b = bs[j]
        bxv[r, :] = box_map[b, :, gy, gx]
        bxt[r, :] = tgt_boxes[b, t]
        h_im, w_im = tgt_sizes[b, 0], tgt_sizes[b, 1]
        bxs[r, :] = np.array([w_im, h_im, w_im, h_im], f32)
        bxw[r, 0] = 1.0

    return dict(hm=hm, lg=lg, sel=sel, cw=cw, srcb=srcb, tgtb=tgtb, sclb=sclb,
                hmx=hmx, hmw=hmw, bxv=bxv, bxt=bxt, bxs=bxs, bxw=bxw)


def fill_missing_inputs(nc, in_maps):
    import concourse.mybir as mybir
    for alloc in nc.m.functions[0].allocations:
        if (isinstance(alloc, mybir.MemoryLocationSet)
                and alloc.kind == "ExternalInput"):
            name = alloc.memorylocations[0].name
            for m in in_maps:
                if name not in m:
                    m[name] = np.zeros(tuple(alloc.tensor_shape),
                                       mybir.dt.np(alloc.dtype))
    return in_maps


def kernel(pred_logits, pred_boxes, heatmap_logits, box_map, tgt_boxes,
           tgt_labels, tgt_sizes, src_idx, tgt_idx, empty_weight):
    global LAST_RESULTS
    from concourse import bass_utils

    pred_logits = np.asarray(pred_logits, np.float32)
    pred_boxes = np.asarray(pred_boxes, np.float32)
    heatmap_logits = np.asarray(heatmap_logits, np.float32)
    box_map = np.asarray(box_map, np.float32)
    tgt_boxes = np.asarray(tgt_boxes, np.float32)
    tgt_labels = np.asarray(tgt_labels)
    tgt_sizes = np.asarray(tgt_sizes, np.float32)
    src_idx = np.asarray(src_idx)
    tgt_idx = np.asarray(tgt_idx)
    empty_weight = np.asarray(empty_weight, np.float32)

    variant = os.environ.get("KERNEL_VARIANT", "v1")
    if ("nc", variant) not in _CACHE:
        _CACHE[("nc", variant)] = _build_module(variant=variant)
    nc = _CACHE[("nc", variant)]

    in_maps = [
        _host_prepare(c, pred_logits, pred_boxes, heatmap_logits, box_map,
                      tgt_boxes, tgt_labels, tgt_sizes, src_idx, tgt_idx,
                      empty_weight)
        for c in range(NCORES)
    ]

    fill_missing_inputs(nc, in_maps)
    res = bass_utils.run_bass_kernel_spmd(
        nc, in_maps, core_ids=list(range(NCORES)))
    LAST_RESULTS = res

    parts = np.stack([res.results[c]["out"][0] for c in range(NCORES)])  # [8, NCOL]
    S = parts.sum(axis=0).astype(np.float32)

    f32 = np.float32
    num_boxes = f32(B * T)
    loss_ce = f32(S[2] / S[3])
    loss_bbox = f32(S[4] / num_boxes)
    loss_giou = f32(S[5] / num_boxes)
    num_pos = max(f32(S[7]), f32(1.0))
    hm_loss = f32((S[0] + S[1]) / num_pos)
    box_loss = f32(S[6] / num_pos)
    loss_aux = f32(AUX_HM_W * hm_loss + AUX_BOX_W * box_loss)
    loss_total = f32(W_CE * loss_ce + W_BBOX * loss_bbox
                     + W_GIOU * loss_giou + AUX_W * loss_aux)
    return np.array([loss_ce, loss_bbox, loss_giou, loss_aux, loss_total],
                    dtype=np.float32)

